# revision 6
# baseline (speedup 1.0000x reference)
"""Trainium2 Bass kernel for nn_MultiMPNN (gnn_message_passing).

Reference computation (B=4, N=512, Z=64, E=16, H=128):
    msgs[b,i,j,:] = z[b,i]@W_i + z[b,j]@W_j + e_feat[b,i,j]@W_e + b_msg
    agg[b,i,:]    = max_j (msgs + (adj>0 ? 0 : -inf))
    out           = z@Wu_z + agg@Wu_h + b_upd

Sharding: 8 cores = (batch b, half of destination rows i).  Each core owns
256 i-rows and the full j axis.

Device-side trick: everything under the max folds into ONE matmul per
(b,i) row with an augmented contraction axis K = E + 1 + Z = 81:
    lhsT_aug[81,128] = [W_e ; -1e9*ones(1,H) ; W_j]      (constant)
    rhs_aug [81,512] = [e_feat[b,i].T ; (1-adj[b,i]) ; z[b].T]
    PSUM[h,j] = ze + mask + zj       ->  reduce_max over j -> agg column
zi + b_msg commute out of the max and are folded into the final linear,
whose z@Wu_z part is computed on the host (tiny, exact f32).
"""

import numpy as np
import ml_dtypes

import concourse.bass as bass
import concourse.bacc as bacc
import concourse.mybir as mybir
import concourse.tile as tile
from concourse import bass_utils
from concourse.bass_interp import get_hw_module
from contextlib import ExitStack

B, N, Z, E, H = 4, 512, 64, 16, 128
NCORES = 8
IH = N * B // NCORES          # 256 destination rows per core
KAUG = E + 1 + Z              # 81
NB = 4                        # rhs ping-pong depth

F32 = mybir.dt.float32
BF16 = mybir.dt.bfloat16
NP_BF16 = ml_dtypes.bfloat16

TRACE = False                 # test.py sets True to capture an NTFF profile
TRACE_DIR = None              # optional fixed dir for trace artifacts
LAST_RESULTS = None           # BassKernelResults of the last run (for test.py)

_MODULE_CACHE = None


def _ensure_ntff_hook():
    """The agent image's antenv lacks axon_hooks; recreate it so
    run_bass_kernel_spmd(trace=True) can reach the axon NTFF profiler."""
    import sys
    import types

    try:
        import antenv.axon_hooks  # noqa: F401

        return
    except ImportError:
        pass
    import antenv
    from trn_agent_boot.trn_boot import _ntff_profile_via_ctypes

    state = {"h": _ntff_profile_via_ctypes("/opt/axon/libaxon_pjrt.so")}
    mod = types.ModuleType("antenv.axon_hooks")
    mod.get_axon_ntff_profile_hook = lambda: state["h"]
    mod.set_axon_ntff_profile_hook = lambda h: state.__setitem__("h", h)
    sys.modules["antenv.axon_hooks"] = mod
    antenv.axon_hooks = mod


def _build_module():
    nc = bacc.Bacc(
        "TRN2",
        target_bir_lowering=False,
        debug=False,
        enable_asserts=False,
        num_devices=NCORES,
    )

    stream = nc.dram_tensor("stream", [IH, E + 1, N], BF16, kind="ExternalInput")
    rhsz = nc.dram_tensor("rhsz", [Z, N], BF16, kind="ExternalInput")
    lhst = nc.dram_tensor("lhst", [KAUG, H], BF16, kind="ExternalInput")
    zit = nc.dram_tensor("zit", [H, IH], F32, kind="ExternalInput")
    hostc = nc.dram_tensor("hostc", [H, IH], F32, kind="ExternalInput")
    wuh = nc.dram_tensor("wuh", [H, H], F32, kind="ExternalInput")
    ident = nc.dram_tensor("ident", [H, H], F32, kind="ExternalInput")
    out = nc.dram_tensor("out", [IH, H], F32, kind="ExternalOutput")

    with ExitStack() as ctx:
        tc = ctx.enter_context(tile.TileContext(nc))
        const = ctx.enter_context(tc.tile_pool(name="const", bufs=1))
        psum = ctx.enter_context(tc.tile_pool(name="psum", bufs=6, space="PSUM"))
        psum2 = ctx.enter_context(tc.tile_pool(name="psum2", bufs=1, space="PSUM"))

        lhst_sb = const.tile([KAUG, H], BF16, tag="lhst")
        nc.sync.dma_start(lhst_sb[:, :], lhst.ap())
        zit_sb = const.tile([H, IH], F32, tag="zit")
        nc.sync.dma_start(zit_sb[:, :], zit.ap())
        hostc_sb = const.tile([H, IH], F32, tag="hostc")
        nc.sync.dma_start(hostc_sb[:, :], hostc.ap())
        wuh_sb = const.tile([H, H], F32, tag="wuh")
        nc.sync.dma_start(wuh_sb[:, :], wuh.ap())
        ident_sb = const.tile([H, H], F32, tag="ident")
        nc.sync.dma_start(ident_sb[:, :], ident.ap())

        rhs_bufs = []
        for k in range(NB):
            rb = const.tile([KAUG, N], BF16, tag=f"rhs{k}")
            nc.sync.dma_start(rb[E + 1 :, :], rhsz.ap())
            rhs_bufs.append(rb)

        magg = const.tile([H, IH], F32, tag="magg")

        stream_ap = stream.ap()
        for i in range(IH):
            rb = rhs_bufs[i % NB]
            nc.sync.dma_start(rb[: E + 1, :], stream_ap[i])
            ps = psum.tile([H, N], F32)
            nc.tensor.matmul(ps[:, :], lhst_sb[:, :], rb[:, :], start=True, stop=True)
            nc.vector.reduce_max(
                magg[:, i : i + 1], ps[:, :], axis=mybir.AxisListType.X
            )

        aggt = const.tile([H, IH], F32, tag="aggt")
        nc.vector.tensor_add(aggt[:, :], magg[:, :], zit_sb[:, :])

        psf = psum2.tile([H, IH], F32, tag="psf")
        nc.tensor.matmul(psf[:, :], wuh_sb[:, :], aggt[:, :], start=True, stop=True)

        outt = const.tile([H, IH], F32, tag="outt")
        nc.vector.tensor_add(outt[:, :], psf[:, :], hostc_sb[:, :])

        out_ap = out.ap()
        for t in range(IH // H):
            pst = psum2.tile([H, H], F32, tag="pst")
            nc.tensor.transpose(
                pst[:, :], outt[:, t * H : (t + 1) * H], ident_sb[:, :]
            )
            osb = const.tile([H, H], F32, tag=f"osb{t}")
            nc.scalar.copy(osb[:, :], pst[:, :])
            nc.sync.dma_start(out_ap[t * H : (t + 1) * H, :], osb[:, :])

    nc.compile()
    nc.m = get_hw_module(nc.m)
    return nc


def kernel(z, e_feat, adj, W_msg, b_msg, W_upd, b_upd):
    global LAST_RESULTS, _MODULE_CACHE

    z = np.asarray(z, np.float32)
    e_feat = np.asarray(e_feat, np.float32)
    adj = np.asarray(adj)
    W_msg = np.asarray(W_msg, np.float32)
    b_msg = np.asarray(b_msg, np.float32)
    W_upd = np.asarray(W_upd, np.float32)
    b_upd = np.asarray(b_upd, np.float32)

    W_i, W_j, W_e = W_msg[:Z], W_msg[Z : 2 * Z], W_msg[2 * Z :]
    Wu_z, Wu_h = W_upd[:Z], W_upd[Z:]

    lhst_np = np.concatenate(
        [W_e, np.full((1, H), -1e9, np.float32), W_j], axis=0
    ).astype(NP_BF16)
    wuh_np = np.ascontiguousarray(Wu_h, np.float32)
    ident_np = np.eye(H, dtype=np.float32)

    in_maps = []
    for c in range(NCORES):
        b, half = divmod(c, NCORES // B)
        sl = slice(half * IH, (half + 1) * IH)
        stream = np.empty((IH, E + 1, N), dtype=NP_BF16)
        stream[:, :E, :] = e_feat[b, sl].transpose(0, 2, 1)
        stream[:, E, :] = (1 - adj[b, sl]).astype(NP_BF16)
        in_maps.append(
            {
                "stream": stream,
                "rhsz": np.ascontiguousarray(z[b].T).astype(NP_BF16),
                "lhst": lhst_np,
                "zit": np.ascontiguousarray(
                    (z[b, sl] @ W_i).T + b_msg[:, None], dtype=np.float32
                ),
                "hostc": np.ascontiguousarray(
                    (z[b, sl] @ Wu_z + b_upd).T, dtype=np.float32
                ),
                "wuh": wuh_np,
                "ident": ident_np,
            }
        )

    if _MODULE_CACHE is None:
        _MODULE_CACHE = _build_module()
    nc = _MODULE_CACHE

    if TRACE:
        _ensure_ntff_hook()
    res = bass_utils.run_bass_kernel_spmd(
        nc, in_maps, core_ids=list(range(NCORES)), trace=TRACE, tmpdir=TRACE_DIR
    )
    LAST_RESULTS = res

    full = np.empty((B, N, H), np.float32)
    for c in range(NCORES):
        b, half = divmod(c, NCORES // B)
        full[b, half * IH : (half + 1) * IH] = res.results[c]["out"]
    return full


if __name__ == "__main__":
    rng = np.random.default_rng(0)
    ins = {
        "z": rng.standard_normal((B, N, Z), np.float32),
        "e_feat": rng.standard_normal((B, N, N, E), np.float32),
        "adj": (rng.random((B, N, N)) < 0.5).astype(np.int32),
        "W_msg": rng.standard_normal((2 * Z + E, H), np.float32) * 0.1,
        "b_msg": np.zeros(H, np.float32),
        "W_upd": rng.standard_normal((Z + H, H), np.float32) * 0.1,
        "b_upd": np.zeros(H, np.float32),
    }
    out = kernel(**ins)
    print("out", out.shape, out.dtype, float(np.abs(out).max()))


# revision 16
# speedup vs baseline: 1.2380x; 1.2380x over previous
"""Trainium2 Bass kernel for nn_MultiMPNN (gnn_message_passing).

Reference computation (B=4, N=512, Z=64, E=16, H=128):
    msgs[b,i,j,:] = z[b,i]@W_i + z[b,j]@W_j + e_feat[b,i,j]@W_e + b_msg
    agg[b,i,:]    = max_j (msgs + (adj>0 ? 0 : -inf))
    out           = z@Wu_z + agg@Wu_h + b_upd

Sharding: 8 cores = (batch b, half of destination rows i).  Each core owns
256 i-rows and the full j axis.

Device-side trick: everything under the max folds into ONE matmul per
(b,i) row with an augmented contraction axis K = E + 1 + Z = 81:
    lhsT_aug[81,128] = [W_e ; -1e9*ones(1,H) ; W_j]      (constant)
    rhs_aug [81,512] = [e_feat[b,i].T ; (1-adj[b,i]) ; z[b].T]
    PSUM[h,j] = ze + mask + zj       ->  reduce_max over j -> agg column
zi + b_msg commute out of the max and are folded into the final linear,
whose z@Wu_z part is computed on the host (tiny, exact f32).
"""

import numpy as np
import ml_dtypes

import concourse.bass as bass
import concourse.bacc as bacc
import concourse.mybir as mybir
import concourse.tile as tile
from concourse import bass_utils
from concourse.bass_interp import get_hw_module
from contextlib import ExitStack

B, N, Z, E, H = 4, 512, 64, 16, 128
NCORES = 8
IH = N * B // NCORES          # 256 destination rows per core
KAUG = E + 1 + Z              # 81
G = 32                        # rows per DMA block
NBLK = IH // G                # 8 blocks per core
RG = 4                        # rows per grouped reduce (PSUM banks per tile)

F32 = mybir.dt.float32
BF16 = mybir.dt.bfloat16
NP_BF16 = ml_dtypes.bfloat16

TRACE = False                 # test.py sets True to capture an NTFF profile
TRACE_DIR = None              # optional fixed dir for trace artifacts
LAST_RESULTS = None           # BassKernelResults of the last run (for test.py)

_MODULE_CACHE = None


def _ensure_ntff_hook():
    """The agent image's antenv lacks axon_hooks; recreate it so
    run_bass_kernel_spmd(trace=True) can reach the axon NTFF profiler."""
    import sys
    import types

    try:
        import antenv.axon_hooks  # noqa: F401

        return
    except ImportError:
        pass
    import antenv
    from trn_agent_boot.trn_boot import _ntff_profile_via_ctypes

    state = {"h": _ntff_profile_via_ctypes("/opt/axon/libaxon_pjrt.so")}
    mod = types.ModuleType("antenv.axon_hooks")
    mod.get_axon_ntff_profile_hook = lambda: state["h"]
    mod.set_axon_ntff_profile_hook = lambda h: state.__setitem__("h", h)
    sys.modules["antenv.axon_hooks"] = mod
    antenv.axon_hooks = mod


def _build_module():
    nc = bacc.Bacc(
        "TRN2",
        target_bir_lowering=False,
        debug=False,
        enable_asserts=False,
        num_devices=NCORES,
    )

    # stream is plane-major: [E+1 planes, IH*N] so each block DMA moves
    # G*N*2 = 32 KiB contiguous per partition.
    stream = nc.dram_tensor("stream", [E + 1, IH * N], BF16, kind="ExternalInput")
    rhsz = nc.dram_tensor("rhsz", [Z, G * N], BF16, kind="ExternalInput")
    lhst = nc.dram_tensor("lhst", [KAUG, H], BF16, kind="ExternalInput")
    zit = nc.dram_tensor("zit", [H, IH], F32, kind="ExternalInput")
    hostc = nc.dram_tensor("hostc", [H, IH], F32, kind="ExternalInput")
    wuh = nc.dram_tensor("wuh", [H, H], F32, kind="ExternalInput")
    ident = nc.dram_tensor("ident", [H, H], F32, kind="ExternalInput")
    out = nc.dram_tensor("out", [IH, H], F32, kind="ExternalOutput")

    with ExitStack() as ctx:
        tc = ctx.enter_context(tile.TileContext(nc))
        const = ctx.enter_context(tc.tile_pool(name="const", bufs=1))
        psum = ctx.enter_context(tc.tile_pool(name="psum", bufs=2, space="PSUM"))

        lhst_sb = const.tile([KAUG, H], BF16, tag="lhst")
        nc.sync.dma_start(lhst_sb[:, :], lhst.ap())
        zit_sb = const.tile([H, IH], F32, tag="zit")
        nc.sync.dma_start(zit_sb[:, :], zit.ap())
        hostc_sb = const.tile([H, IH], F32, tag="hostc")
        nc.sync.dma_start(hostc_sb[:, :], hostc.ap())
        wuh_sb = const.tile([H, H], F32, tag="wuh")
        nc.sync.dma_start(wuh_sb[:, :], wuh.ap())
        ident_sb = const.tile([H, H], F32, tag="ident")
        nc.sync.dma_start(ident_sb[:, :], ident.ap())

        mega_bufs = []
        for k in range(2):
            mb = const.tile([KAUG, G * N], BF16, tag=f"mega{k}")
            nc.sync.dma_start(mb[E + 1 :, :], rhsz.ap())
            mega_bufs.append(mb)

        magg = const.tile([H, IH], F32, tag="magg")

        stream_ap = stream.ap()
        for blk in range(NBLK):
            mb = mega_bufs[blk % 2]
            nc.sync.dma_start(
                mb[: E + 1, :], stream_ap[:, blk * G * N : (blk + 1) * G * N]
            )
            for g4 in range(G // RG):
                ps = psum.tile([H, RG * N], F32, tag="ps")
                for r in range(RG):
                    g = g4 * RG + r
                    nc.tensor.matmul(
                        ps[:, r * N : (r + 1) * N],
                        lhst_sb[:, :],
                        mb[:, g * N : (g + 1) * N],
                        start=True,
                        stop=True,
                    )
                i0 = blk * G + g4 * RG
                nc.vector.reduce_max(
                    magg[:, i0 : i0 + RG],
                    ps[:, :].rearrange("p (g j) -> p g j", g=RG),
                    axis=mybir.AxisListType.X,
                )

        aggt = const.tile([H, IH], F32, tag="aggt")
        nc.vector.tensor_add(aggt[:, :], magg[:, :], zit_sb[:, :])

        psf = psum.tile([H, RG * N], F32, tag="ps")
        nc.tensor.matmul(
            psf[:, :IH], wuh_sb[:, :], aggt[:, :], start=True, stop=True
        )

        outt = const.tile([H, IH], F32, tag="outt")
        nc.vector.tensor_add(outt[:, :], psf[:, :IH], hostc_sb[:, :])

        out_ap = out.ap()
        for t in range(IH // H):
            pst = psum.tile([H, RG * N], F32, tag="ps")
            nc.tensor.transpose(
                pst[:, :H], outt[:, t * H : (t + 1) * H], ident_sb[:, :]
            )
            osb = const.tile([H, H], F32, tag=f"osb{t}")
            nc.scalar.copy(osb[:, :], pst[:, :H])
            nc.sync.dma_start(out_ap[t * H : (t + 1) * H, :], osb[:, :])

    nc.compile()
    nc.m = get_hw_module(nc.m)
    return nc


def kernel(z, e_feat, adj, W_msg, b_msg, W_upd, b_upd):
    global LAST_RESULTS, _MODULE_CACHE

    z = np.asarray(z, np.float32)
    e_feat = np.asarray(e_feat, np.float32)
    adj = np.asarray(adj)
    W_msg = np.asarray(W_msg, np.float32)
    b_msg = np.asarray(b_msg, np.float32)
    W_upd = np.asarray(W_upd, np.float32)
    b_upd = np.asarray(b_upd, np.float32)

    W_i, W_j, W_e = W_msg[:Z], W_msg[Z : 2 * Z], W_msg[2 * Z :]
    Wu_z, Wu_h = W_upd[:Z], W_upd[Z:]

    lhst_np = np.concatenate(
        [W_e, np.full((1, H), -1e9, np.float32), W_j], axis=0
    ).astype(NP_BF16)
    wuh_np = np.ascontiguousarray(Wu_h, np.float32)
    ident_np = np.eye(H, dtype=np.float32)

    in_maps = []
    for c in range(NCORES):
        b, half = divmod(c, NCORES // B)
        sl = slice(half * IH, (half + 1) * IH)
        stream = np.empty((E + 1, IH, N), dtype=NP_BF16)
        stream[:E] = e_feat[b, sl].transpose(2, 0, 1)
        stream[E] = (1 - adj[b, sl]).astype(NP_BF16)
        in_maps.append(
            {
                "stream": stream.reshape(E + 1, IH * N),
                "rhsz": np.tile(
                    np.ascontiguousarray(z[b].T).astype(NP_BF16), (1, G)
                ),
                "lhst": lhst_np,
                "zit": np.ascontiguousarray(
                    (z[b, sl] @ W_i).T + b_msg[:, None], dtype=np.float32
                ),
                "hostc": np.ascontiguousarray(
                    (z[b, sl] @ Wu_z + b_upd).T, dtype=np.float32
                ),
                "wuh": wuh_np,
                "ident": ident_np,
            }
        )

    if _MODULE_CACHE is None:
        _MODULE_CACHE = _build_module()
    nc = _MODULE_CACHE

    if TRACE:
        _ensure_ntff_hook()
    res = bass_utils.run_bass_kernel_spmd(
        nc, in_maps, core_ids=list(range(NCORES)), trace=TRACE, tmpdir=TRACE_DIR
    )
    LAST_RESULTS = res

    full = np.empty((B, N, H), np.float32)
    for c in range(NCORES):
        b, half = divmod(c, NCORES // B)
        full[b, half * IH : (half + 1) * IH] = res.results[c]["out"]
    return full


if __name__ == "__main__":
    rng = np.random.default_rng(0)
    ins = {
        "z": rng.standard_normal((B, N, Z), np.float32),
        "e_feat": rng.standard_normal((B, N, N, E), np.float32),
        "adj": (rng.random((B, N, N)) < 0.5).astype(np.int32),
        "W_msg": rng.standard_normal((2 * Z + E, H), np.float32) * 0.1,
        "b_msg": np.zeros(H, np.float32),
        "W_upd": rng.standard_normal((Z + H, H), np.float32) * 0.1,
        "b_upd": np.zeros(H, np.float32),
    }
    out = kernel(**ins)
    print("out", out.shape, out.dtype, float(np.abs(out).max()))
